# revision 13
# baseline (speedup 1.0000x reference)
"""MoE (N=16384, D=512, E=8, top_k=2) on 8 trn2 NeuronCores.

Strategy: group tokens globally by their unordered expert pair {e_a, e_b}
(28 groups for E=8), shard every group evenly across the 8 cores (96 slots
per core-group segment). Each core runs an identical (SPMD) program: 28
tiles of 96 tokens, each doing 8 accumulating float32r matmuls (2 experts
x 4 K-chunks, moving free dim 512) into two PSUM banks, then a
per-partition gate-weighted combine split across the Scalar and Vector
engines. All routing data-dependence lives in the host-side input
arrangement; the device program is fixed.
"""

import numpy as np

# ---------------------------------------------------------------------------
# The walrus build in this image accepts at most ONE sync-wait command per
# instruction, while Tile's semaphore assignment attaches several (DMA WAR +
# producer sems, and the kernel-tail drain waits on every live proc). Post-
# pass over the finished BIR: any instruction carrying more than one wait is
# preceded by same-engine nops that each take one wait. The engine executes
# its queue in order, so semantics are unchanged.
import bass_rust

_MAX_WAITS = 1


def _split_multi_waits(nc):
    for f in nc.m.functions:
        for blk in f.blocks:
            insts = blk.instructions
            k = 0
            while k < len(insts):
                inst = insts[k]
                si = getattr(inst, "sync_info", None)
                if si is not None and si.on_wait and len(si.on_wait) > _MAX_WAITS:
                    waits = list(si.on_wait)
                    keep = waits[-_MAX_WAITS:]
                    extra = waits[:-_MAX_WAITS]
                    inst.sync_info = bass_rust.SyncInfo(
                        on_wait=keep, on_update=list(si.on_update)
                    )
                    for j, i0 in enumerate(range(0, len(extra), _MAX_WAITS)):
                        nop = bass_rust.InstNoOp(
                            name=f"{inst.name}-wsplit{j}", ins=[], outs=[]
                        )
                        nop.engine = inst.engine
                        nop.sync_info = bass_rust.SyncInfo(
                            on_wait=extra[i0 : i0 + _MAX_WAITS], on_update=[]
                        )
                        insts.insert(k, nop)
                        k += 1
                k += 1
# ---------------------------------------------------------------------------

# Re-enable walrus's LDWEIGHTS dedup/pipelining pass: fp32r matmuls self-load
# their stationary operand, which would otherwise serialize with every
# matmul stream.
import concourse.bass_utils as _bu

if not getattr(_bu, "_ldw_opt_patched", False):
    _orig_run_command = _bu.run_command

    def _run_command_ldw(cmd, **kw):
        cmd = [
            "--enable-ldw-opt=true" if c == "--enable-ldw-opt=false" else c
            for c in cmd
        ]
        return _orig_run_command(cmd, **kw)

    _bu.run_command = _run_command_ldw
    _bu._ldw_opt_patched = True

import concourse.bass as bass
import concourse.mybir as mybir
from concourse.tile import TileContext
from concourse.bass_utils import run_bass_kernel_spmd

N, D, E, TOPK = 16384, 512, 8, 2
NCORES = 8
# triangle order: pair {a,b} (a<b) sorted by max expert, so expert m is
# first needed late and its weights can stream in during compute
PAIRS = [(a, b) for b in range(E) for a in range(b)]  # canonical order
G = len(PAIRS)  # 28
GCAP = 96  # token slots per (core, group) tile
ROWS = G * GCAP  # 2688 rows per core
KCH = D // 128  # 4 contraction chunks
XBLK = 896  # token-columns per x-load block
NXB = ROWS // XBLK  # 3

LAST_EXEC_TIME_NS = None  # set by kernel() when tracing is active

_cache = {}


def _build_bass():
    f32 = mybir.dt.float32
    f32r = mybir.dt.float32r
    nc = bass.Bass()
    # xh: [p, (blk, kc, col)] with 896-column blocks -> 3.5KB DMA runs
    # wdma: [p, (e, kc, dout)] -> 8KB DMA runs, one DMA per expert
    xh = nc.declare_dram_parameter("xh", [128, ROWS * KCH], f32r, isOutput=False)
    pwa = nc.declare_dram_parameter("pwa", [GCAP, G * 2], f32, isOutput=False)
    wdma = nc.declare_dram_parameter(
        "wdma", [128, E * KCH * D], f32r, isOutput=False
    )
    y = nc.declare_dram_parameter("y", [ROWS, D], f32, isOutput=True)

    with TileContext(nc) as tc:
        with (
            tc.tile_pool(name="const", bufs=1) as cpool,
            tc.tile_pool(name="vpool", bufs=3) as vpool,
            tc.tile_pool(name="opool", bufs=3) as opool,
            tc.tile_pool(name="psum", bufs=3, space="PSUM") as pspool,
        ):
            # Gate weights for every group tile: [row, group*2], one DMA.
            pw_all = cpool.tile([GCAP, G * 2], f32)

            # Whole x^T shard stays resident in SBUF: 4 contraction-chunk
            # tiles of [128, ROWS]. Expert weights likewise, one tile per
            # expert. Loads are interleaved so tile 0's operands arrive
            # first and compute overlaps the remaining streams.
            xsb = [
                cpool.tile([128, ROWS], f32r, tag=f"x{kc}", name=f"xsb{kc}")
                for kc in range(KCH)
            ]
            w_tiles = [
                cpool.tile([128, KCH * D], f32r, tag=f"w{e}", name=f"wsb{e}")
                for e in range(E)
            ]

            def load_w(e):
                nc.sync.dma_start(
                    w_tiles[e][:], wdma[:, e * KCH * D : (e + 1) * KCH * D]
                )

            def load_x(blk):
                for kc in range(KCH):
                    nc.sync.dma_start(
                        xsb[kc][:, blk * XBLK : (blk + 1) * XBLK],
                        xh[:, (blk * KCH + kc) * XBLK : (blk * KCH + kc + 1) * XBLK],
                    )

            # PE warmup: the HAM clock gate defaults to 1.2 GHz and only
            # releases after ~3.4us of sustained PE activity. Run dummy
            # matmuls on a scratch tile during the initial DMA window so
            # the real matmuls start at 2.4 GHz.
            scratch = cpool.tile([128, 512], f32r)
            nc.sync.dma_start(scratch[:], wdma[:, 0:512])
            warm_ps = pspool.tile([128, D], f32, tag="warm", bufs=1)
            for _ in range(18):
                nc.tensor.matmul(
                    warm_ps[:], scratch[:, 0:128], scratch[:], start=True, stop=True
                )

            # fast start: tile 0 only needs W0.kc0, W1.kc0 and x cols 0:GCAP;
            # stream everything else behind it
            def load_w_split(e):
                for kc in range(KCH):
                    nc.sync.dma_start(
                        w_tiles[e][:, kc * D : (kc + 1) * D],
                        wdma[:, (e * KCH + kc) * D : (e * KCH + kc + 1) * D],
                    )

            load_w_split(0)
            load_w_split(1)
            for kc in range(KCH):
                nc.sync.dma_start(
                    xsb[kc][:, 0:GCAP], xh[:, kc * XBLK : kc * XBLK + GCAP]
                )
            nc.sync.dma_start(pw_all[:], pwa[:, :])
            for kc in range(KCH):
                nc.sync.dma_start(
                    xsb[kc][:, GCAP:XBLK],
                    xh[:, kc * XBLK + GCAP : (kc + 1) * XBLK],
                )
            load_w(2)
            load_w(3)
            load_w(4)
            load_x(1)
            load_w(5)
            load_w(6)
            load_w(7)
            load_x(2)

            for g, (a, b) in enumerate(PAIRS):
                pa = pspool.tile([GCAP, D], f32, tag="pa")
                pb = pspool.tile([GCAP, D], f32, tag="pb")
                for kc in range(KCH):
                    # both experts consume the same stationary x chunk
                    # back-to-back so walrus's ldw-opt can skip the reload
                    nc.tensor.matmul(
                        pa[:],
                        xsb[kc][:, g * GCAP : (g + 1) * GCAP],
                        w_tiles[a][:, kc * D : (kc + 1) * D],
                        start=(kc == 0),
                        stop=(kc == KCH - 1),
                    )
                    nc.tensor.matmul(
                        pb[:],
                        xsb[kc][:, g * GCAP : (g + 1) * GCAP],
                        w_tiles[b][:, kc * D : (kc + 1) * D],
                        start=(kc == 0),
                        stop=(kc == KCH - 1),
                    )
                # combine: out = pa*w_lo + pb*w_hi, split across ACT and DVE
                tmp = vpool.tile([GCAP, D], f32)
                nc.scalar.activation(
                    tmp[:],
                    pb[:],
                    mybir.ActivationFunctionType.Copy,
                    scale=pw_all[:, 2 * g + 1 : 2 * g + 2],
                )
                o = opool.tile([GCAP, D], f32)
                nc.vector.scalar_tensor_tensor(
                    o[:],
                    pa[:],
                    pw_all[:, 2 * g : 2 * g + 1],
                    tmp[:],
                    mybir.AluOpType.mult,
                    mybir.AluOpType.add,
                )
                nc.gpsimd.dma_start(y[g * GCAP : (g + 1) * GCAP, :], o[:])
    _split_multi_waits(nc)
    return nc


def _assign(indices, probabilities):
    """Build per-core row assignments for every (token, gate) pair.

    Returns rows[c] = list of (token, group, w_lo, w_hi). Normal path:
    each token appears exactly once (both its gates land in the group of
    its expert pair). Overflow/duplicate-expert fallbacks split a token
    into two single-gate rows placed in any group containing that expert.
    """
    gid = {p: g for g, p in enumerate(PAIRS)}
    idx0, idx1 = indices[:, 0].astype(np.int64), indices[:, 1].astype(np.int64)
    p0, p1 = probabilities[:, 0], probabilities[:, 1]
    lo = np.minimum(idx0, idx1)
    hi = np.maximum(idx0, idx1)
    w_lo = np.where(idx0 <= idx1, p0, p1)
    w_hi = np.where(idx0 <= idx1, p1, p0)

    entries = [[] for _ in range(G)]  # group -> list of (token, w_lo, w_hi)
    singles = []  # (token, expert, weight) fallback entries
    dup = lo == hi
    for n in np.nonzero(dup)[0]:
        singles.append((int(n), int(lo[n]), float(p0[n] + p1[n])))
    ok = np.nonzero(~dup)[0]
    gids = np.array([gid[(int(a), int(b))] for a, b in zip(lo[ok], hi[ok])])
    for g in range(G):
        for n in ok[gids == g]:
            entries[g].append((int(n), float(w_lo[n]), float(w_hi[n])))

    rows = [[] for _ in range(NCORES)]  # core -> (token, group, wl, wh)
    used = np.zeros((NCORES, G), np.int64)
    for g in range(G):
        for j, (n, wl, wh) in enumerate(entries[g]):
            c = j % NCORES
            if used[c, g] < GCAP:
                rows[c].append((n, g, wl, wh))
                used[c, g] += 1
            else:
                a, b = PAIRS[g]
                singles.append((n, a, wl))
                singles.append((n, b, wh))
    for n, e, w in singles:
        placed = False
        for c in range(NCORES):
            for g in range(G):
                if used[c, g] < GCAP and e in PAIRS[g]:
                    a, b = PAIRS[g]
                    wl, wh = (w, 0.0) if e == a else (0.0, w)
                    rows[c].append((n, g, wl, wh))
                    used[c, g] += 1
                    placed = True
                    break
            if placed:
                break
        assert placed, "no capacity left for fallback entry"
    return rows


def kernel(input_batch, probabilities, indices, W, b, **_unused):
    global LAST_EXEC_TIME_NS
    x = np.ascontiguousarray(np.asarray(input_batch, dtype=np.float32))
    p = np.ascontiguousarray(np.asarray(probabilities, dtype=np.float32))
    idx = np.asarray(indices)
    Wf = np.ascontiguousarray(np.asarray(W, dtype=np.float32))
    bf = np.asarray(b, dtype=np.float32)
    assert x.shape == (N, D) and p.shape == (N, TOPK)
    assert idx.shape == (N, TOPK) and Wf.shape == (E, D, D)

    rows = _assign(idx, p)

    # [p, (e, kc, dout)] layout; see _build_bass
    wdma = np.ascontiguousarray(
        Wf.reshape(E, KCH, 128, D).transpose(2, 0, 1, 3).reshape(128, E * KCH * D)
    )

    in_maps = []
    tok_maps = []
    for c in range(NCORES):
        x_rows = np.zeros((ROWS, D), np.float32)
        pw_arr = np.zeros((ROWS, 2), np.float32)
        tok_arr = np.full(ROWS, -1, np.int64)
        slot_used = np.zeros(G, np.int64)
        for n, g, wl, wh in rows[c]:
            s = g * GCAP + slot_used[g]
            slot_used[g] += 1
            x_rows[s] = x[n]
            pw_arr[s, 0] = wl
            pw_arr[s, 1] = wh
            tok_arr[s] = n
        # [p, (blk, kc, col)] layout; see _build_bass
        xh = (
            x_rows.reshape(NXB, XBLK, KCH, 128)
            .transpose(3, 0, 2, 1)
            .reshape(128, ROWS * KCH)
        )
        pwa = (
            pw_arr.reshape(G, GCAP, 2).transpose(1, 0, 2).reshape(GCAP, G * 2)
        )
        in_maps.append(
            {
                "xh": np.ascontiguousarray(xh),
                "pwa": np.ascontiguousarray(pwa),
                "wdma": wdma,
            }
        )
        tok_maps.append(tok_arr)

    if "nc" not in _cache:
        _cache["nc"] = _build_bass()
    nc = _cache["nc"]

    res = run_bass_kernel_spmd(nc, in_maps, list(range(NCORES)))
    LAST_EXEC_TIME_NS = res.exec_time_ns

    out = np.zeros((N, D), np.float32)
    all_tok = np.concatenate(tok_maps)
    all_y = np.concatenate([res.results[c]["y"] for c in range(NCORES)], axis=0)
    valid = all_tok >= 0
    vt = all_tok[valid]
    counts = np.bincount(vt, minlength=N)
    if counts.max() <= 1:
        out[vt] = all_y[valid]
    else:
        np.add.at(out, vt, all_y[valid])

    if np.any(bf):
        # gate-weighted bias: out[n] += sum_k p[n,k] * b[idx[n,k]]
        mask = np.zeros((N, E), np.float32)
        np.add.at(mask, (np.arange(N)[:, None], idx.astype(np.int64)), p)
        out += mask @ bf

    total_loss = np.float32(0.0)
    return out, total_loss


# revision 15
# speedup vs baseline: 1.0230x; 1.0230x over previous
"""MoE (N=16384, D=512, E=8, top_k=2) on 8 trn2 NeuronCores.

Strategy: group tokens globally by their unordered expert pair {e_a, e_b}
(28 groups for E=8), shard every group evenly across the 8 cores (96 slots
per core-group segment). Each core runs an identical (SPMD) program: 28
tiles of 96 tokens, each doing 8 accumulating float32r matmuls (2 experts
x 4 K-chunks, moving free dim 512) into two PSUM banks, then a
per-partition gate-weighted combine split across the Scalar and Vector
engines. All routing data-dependence lives in the host-side input
arrangement; the device program is fixed.
"""

import numpy as np

# ---------------------------------------------------------------------------
# The walrus build in this image accepts at most ONE sync-wait command per
# instruction, while Tile's semaphore assignment attaches several (DMA WAR +
# producer sems, and the kernel-tail drain waits on every live proc). Post-
# pass over the finished BIR: any instruction carrying more than one wait is
# preceded by same-engine nops that each take one wait. The engine executes
# its queue in order, so semantics are unchanged.
import bass_rust

_MAX_WAITS = 1


def _split_multi_waits(nc):
    for f in nc.m.functions:
        for blk in f.blocks:
            insts = blk.instructions
            k = 0
            while k < len(insts):
                inst = insts[k]
                si = getattr(inst, "sync_info", None)
                if si is not None and si.on_wait and len(si.on_wait) > _MAX_WAITS:
                    waits = list(si.on_wait)
                    keep = waits[-_MAX_WAITS:]
                    extra = waits[:-_MAX_WAITS]
                    inst.sync_info = bass_rust.SyncInfo(
                        on_wait=keep, on_update=list(si.on_update)
                    )
                    for j, i0 in enumerate(range(0, len(extra), _MAX_WAITS)):
                        nop = bass_rust.InstNoOp(
                            name=f"{inst.name}-wsplit{j}", ins=[], outs=[]
                        )
                        nop.engine = inst.engine
                        nop.sync_info = bass_rust.SyncInfo(
                            on_wait=extra[i0 : i0 + _MAX_WAITS], on_update=[]
                        )
                        insts.insert(k, nop)
                        k += 1
                k += 1
# ---------------------------------------------------------------------------

# Re-enable walrus's LDWEIGHTS dedup/pipelining pass: fp32r matmuls self-load
# their stationary operand, which would otherwise serialize with every
# matmul stream.
import concourse.bass_utils as _bu

if not getattr(_bu, "_ldw_opt_patched", False):
    _orig_run_command = _bu.run_command

    def _run_command_ldw(cmd, **kw):
        cmd = [
            "--enable-ldw-opt=true" if c == "--enable-ldw-opt=false" else c
            for c in cmd
        ]
        return _orig_run_command(cmd, **kw)

    _bu.run_command = _run_command_ldw
    _bu._ldw_opt_patched = True

import concourse.bass as bass
import concourse.mybir as mybir
from concourse.tile import TileContext
from concourse.bass_utils import run_bass_kernel_spmd

N, D, E, TOPK = 16384, 512, 8, 2
NCORES = 8
# triangle order: pair {a,b} (a<b) sorted by max expert, so expert m is
# first needed late and its weights can stream in during compute
PAIRS = [(a, b) for b in range(E) for a in range(b)]  # canonical order
G = len(PAIRS)  # 28
GCAP = 96  # token slots per (core, group) tile
ROWS = G * GCAP  # 2688 rows per core
KCH = D // 128  # 4 contraction chunks
XBLK = 896  # token-columns per x-load block
NXB = ROWS // XBLK  # 3

LAST_EXEC_TIME_NS = None  # set by kernel() when tracing is active

_cache = {}


def _build_bass():
    f32 = mybir.dt.float32
    f32r = mybir.dt.float32r
    nc = bass.Bass()
    # xh: [p, (blk, kc, col)] with 896-column blocks -> 3.5KB DMA runs
    # wdma: [p, (e, kc, dout)] -> 8KB DMA runs, one DMA per expert
    xh = nc.declare_dram_parameter("xh", [128, ROWS * KCH], f32r, isOutput=False)
    pwa = nc.declare_dram_parameter("pwa", [GCAP, G * 2], f32, isOutput=False)
    wdma = nc.declare_dram_parameter(
        "wdma", [128, E * KCH * D], f32r, isOutput=False
    )
    y = nc.declare_dram_parameter("y", [ROWS, D], f32, isOutput=True)

    with TileContext(nc) as tc:
        with (
            tc.tile_pool(name="const", bufs=1) as cpool,
            tc.tile_pool(name="vpool", bufs=3) as vpool,
            tc.tile_pool(name="opool", bufs=3) as opool,
            tc.tile_pool(name="psum", bufs=3, space="PSUM") as pspool,
        ):
            # Gate weights for every group tile: [row, group*2], one DMA.
            pw_all = cpool.tile([GCAP, G * 2], f32)

            # Whole x^T shard stays resident in SBUF: 4 contraction-chunk
            # tiles of [128, ROWS]. Expert weights likewise, one tile per
            # expert. Loads are interleaved so tile 0's operands arrive
            # first and compute overlaps the remaining streams.
            xsb = [
                cpool.tile([128, ROWS], f32r, tag=f"x{kc}", name=f"xsb{kc}")
                for kc in range(KCH)
            ]
            w_tiles = [
                cpool.tile([128, KCH * D], f32r, tag=f"w{e}", name=f"wsb{e}")
                for e in range(E)
            ]

            def load_w(e):
                nc.sync.dma_start(
                    w_tiles[e][:], wdma[:, e * KCH * D : (e + 1) * KCH * D]
                )

            def load_x(blk):
                for kc in range(KCH):
                    nc.sync.dma_start(
                        xsb[kc][:, blk * XBLK : (blk + 1) * XBLK],
                        xh[:, (blk * KCH + kc) * XBLK : (blk * KCH + kc + 1) * XBLK],
                    )

            # PE warmup: the HAM clock gate defaults to 1.2 GHz and only
            # releases after ~3.4us of sustained PE activity. Run dummy
            # matmuls on a scratch tile during the initial DMA window so
            # the real matmuls start at 2.4 GHz. The scratch DMA is tiny
            # (32KB) so warmup starts almost immediately.
            scratch = cpool.tile([128, 64], f32r)
            nc.sync.dma_start(scratch[:], wdma[:, 0:64])
            warm_ps = pspool.tile([128, D], f32, tag="warm", bufs=1)
            for _ in range(56):
                nc.tensor.matmul(
                    warm_ps[0:64, 0:64], scratch[:], scratch[:], start=True, stop=True
                )

            def load_w_kc(e, kc):
                nc.sync.dma_start(
                    w_tiles[e][:, kc * D : (kc + 1) * D],
                    wdma[:, (e * KCH + kc) * D : (e * KCH + kc + 1) * D],
                )

            # loads in exact first-use order: tile 0 consumes
            # (W0.kc, W1.kc, x.kc[0:GCAP]) for kc = 0..3, then pw for the
            # first combine, then the remaining experts/blocks interleaved
            # in the order the group loop first touches them.
            for kc in range(KCH):
                load_w_kc(0, kc)
                load_w_kc(1, kc)
                nc.sync.dma_start(
                    xsb[kc][:, 0:GCAP], xh[:, kc * XBLK : kc * XBLK + GCAP]
                )
            nc.sync.dma_start(pw_all[:], pwa[:, :])
            for kc in range(KCH):
                nc.sync.dma_start(
                    xsb[kc][:, GCAP:XBLK],
                    xh[:, kc * XBLK + GCAP : (kc + 1) * XBLK],
                )
            load_w(2)
            load_w(3)
            load_w(4)
            load_x(1)
            load_w(5)
            load_w(6)
            load_x(2)
            load_w(7)

            for g, (a, b) in enumerate(PAIRS):
                pa = pspool.tile([GCAP, D], f32, tag="pa")
                pb = pspool.tile([GCAP, D], f32, tag="pb")
                for kc in range(KCH):
                    # both experts consume the same stationary x chunk
                    # back-to-back so walrus's ldw-opt can skip the reload
                    nc.tensor.matmul(
                        pa[:],
                        xsb[kc][:, g * GCAP : (g + 1) * GCAP],
                        w_tiles[a][:, kc * D : (kc + 1) * D],
                        start=(kc == 0),
                        stop=(kc == KCH - 1),
                    )
                    nc.tensor.matmul(
                        pb[:],
                        xsb[kc][:, g * GCAP : (g + 1) * GCAP],
                        w_tiles[b][:, kc * D : (kc + 1) * D],
                        start=(kc == 0),
                        stop=(kc == KCH - 1),
                    )
                # combine: out = pa*w_lo + pb*w_hi, split across ACT and DVE
                tmp = vpool.tile([GCAP, D], f32)
                nc.scalar.activation(
                    tmp[:],
                    pb[:],
                    mybir.ActivationFunctionType.Copy,
                    scale=pw_all[:, 2 * g + 1 : 2 * g + 2],
                )
                o = opool.tile([GCAP, D], f32)
                nc.vector.scalar_tensor_tensor(
                    o[:],
                    pa[:],
                    pw_all[:, 2 * g : 2 * g + 1],
                    tmp[:],
                    mybir.AluOpType.mult,
                    mybir.AluOpType.add,
                )
                nc.gpsimd.dma_start(y[g * GCAP : (g + 1) * GCAP, :], o[:])
    _split_multi_waits(nc)
    return nc


def _assign(indices, probabilities):
    """Build per-core row assignments for every (token, gate) pair.

    Returns rows[c] = list of (token, group, w_lo, w_hi). Normal path:
    each token appears exactly once (both its gates land in the group of
    its expert pair). Overflow/duplicate-expert fallbacks split a token
    into two single-gate rows placed in any group containing that expert.
    """
    gid = {p: g for g, p in enumerate(PAIRS)}
    idx0, idx1 = indices[:, 0].astype(np.int64), indices[:, 1].astype(np.int64)
    p0, p1 = probabilities[:, 0], probabilities[:, 1]
    lo = np.minimum(idx0, idx1)
    hi = np.maximum(idx0, idx1)
    w_lo = np.where(idx0 <= idx1, p0, p1)
    w_hi = np.where(idx0 <= idx1, p1, p0)

    entries = [[] for _ in range(G)]  # group -> list of (token, w_lo, w_hi)
    singles = []  # (token, expert, weight) fallback entries
    dup = lo == hi
    for n in np.nonzero(dup)[0]:
        singles.append((int(n), int(lo[n]), float(p0[n] + p1[n])))
    ok = np.nonzero(~dup)[0]
    gids = np.array([gid[(int(a), int(b))] for a, b in zip(lo[ok], hi[ok])])
    for g in range(G):
        for n in ok[gids == g]:
            entries[g].append((int(n), float(w_lo[n]), float(w_hi[n])))

    rows = [[] for _ in range(NCORES)]  # core -> (token, group, wl, wh)
    used = np.zeros((NCORES, G), np.int64)
    for g in range(G):
        for j, (n, wl, wh) in enumerate(entries[g]):
            c = j % NCORES
            if used[c, g] < GCAP:
                rows[c].append((n, g, wl, wh))
                used[c, g] += 1
            else:
                a, b = PAIRS[g]
                singles.append((n, a, wl))
                singles.append((n, b, wh))
    for n, e, w in singles:
        placed = False
        for c in range(NCORES):
            for g in range(G):
                if used[c, g] < GCAP and e in PAIRS[g]:
                    a, b = PAIRS[g]
                    wl, wh = (w, 0.0) if e == a else (0.0, w)
                    rows[c].append((n, g, wl, wh))
                    used[c, g] += 1
                    placed = True
                    break
            if placed:
                break
        assert placed, "no capacity left for fallback entry"
    return rows


def kernel(input_batch, probabilities, indices, W, b, **_unused):
    global LAST_EXEC_TIME_NS
    x = np.ascontiguousarray(np.asarray(input_batch, dtype=np.float32))
    p = np.ascontiguousarray(np.asarray(probabilities, dtype=np.float32))
    idx = np.asarray(indices)
    Wf = np.ascontiguousarray(np.asarray(W, dtype=np.float32))
    bf = np.asarray(b, dtype=np.float32)
    assert x.shape == (N, D) and p.shape == (N, TOPK)
    assert idx.shape == (N, TOPK) and Wf.shape == (E, D, D)

    rows = _assign(idx, p)

    # [p, (e, kc, dout)] layout; see _build_bass
    wdma = np.ascontiguousarray(
        Wf.reshape(E, KCH, 128, D).transpose(2, 0, 1, 3).reshape(128, E * KCH * D)
    )

    in_maps = []
    tok_maps = []
    for c in range(NCORES):
        x_rows = np.zeros((ROWS, D), np.float32)
        pw_arr = np.zeros((ROWS, 2), np.float32)
        tok_arr = np.full(ROWS, -1, np.int64)
        slot_used = np.zeros(G, np.int64)
        for n, g, wl, wh in rows[c]:
            s = g * GCAP + slot_used[g]
            slot_used[g] += 1
            x_rows[s] = x[n]
            pw_arr[s, 0] = wl
            pw_arr[s, 1] = wh
            tok_arr[s] = n
        # [p, (blk, kc, col)] layout; see _build_bass
        xh = (
            x_rows.reshape(NXB, XBLK, KCH, 128)
            .transpose(3, 0, 2, 1)
            .reshape(128, ROWS * KCH)
        )
        pwa = (
            pw_arr.reshape(G, GCAP, 2).transpose(1, 0, 2).reshape(GCAP, G * 2)
        )
        in_maps.append(
            {
                "xh": np.ascontiguousarray(xh),
                "pwa": np.ascontiguousarray(pwa),
                "wdma": wdma,
            }
        )
        tok_maps.append(tok_arr)

    if "nc" not in _cache:
        _cache["nc"] = _build_bass()
    nc = _cache["nc"]

    res = run_bass_kernel_spmd(nc, in_maps, list(range(NCORES)))
    LAST_EXEC_TIME_NS = res.exec_time_ns

    out = np.zeros((N, D), np.float32)
    all_tok = np.concatenate(tok_maps)
    all_y = np.concatenate([res.results[c]["y"] for c in range(NCORES)], axis=0)
    valid = all_tok >= 0
    vt = all_tok[valid]
    counts = np.bincount(vt, minlength=N)
    if counts.max() <= 1:
        out[vt] = all_y[valid]
    else:
        np.add.at(out, vt, all_y[valid])

    if np.any(bf):
        # gate-weighted bias: out[n] += sum_k p[n,k] * b[idx[n,k]]
        mask = np.zeros((N, E), np.float32)
        np.add.at(mask, (np.arange(N)[:, None], idx.astype(np.int64)), p)
        out += mask @ bf

    total_loss = np.float32(0.0)
    return out, total_loss
